# revision 1
# baseline (speedup 1.0000x reference)
"""Trainium2 Bass kernel for nn_EncoderRNN (batched GRU-step encoder).

Math (see derivation below): the reference is
    emb = x @ w_emb.T + b_emb
    gi  = emb @ w_ih.T + b_ih
    r   = sigmoid(gi_r + b_hr); z = sigmoid(gi_z + b_hz)
    n   = tanh(gi_n + r * b_hn)
    h   = (1 - z) * n
Both matmuls are linear, so they fold into one:  gi = x @ (w_ih @ w_emb).T + (w_ih @ b_emb + b_ih).
Additionally 1 - sigmoid(a) = sigmoid(-a), so the z-gate block of the fused
weight/bias is negated on the host and the device computes
    h = sigmoid(gi_zneg) * tanh(gi_n + sigmoid(gi_r) * b_hn)
with a single K=128 matmul per token tile.

Distribution: pure data parallel over the batch dim, 8 NeuronCores,
16 batches (8192 tokens) per core.  Weights are replicated.
"""

import numpy as np

B, S, I, Hd = 128, 512, 128, 512
G3 = 3 * Hd
N_CORES = 8
B_PER_CORE = B // N_CORES            # 16
TOK = B_PER_CORE * S                 # 8192 tokens per core
GROUP_TOK = 512                      # tokens per group (4 tiles of 128)
J = GROUP_TOK // 128                 # 4 tiles per group
N_GROUPS = TOK // GROUP_TOK          # 16 groups per core

_compiled = {}


def _build_program():
    import concourse.bacc as bacc
    import concourse.tile as tile
    from concourse import mybir
    from concourse.masks import make_identity

    F32 = mybir.dt.float32
    AF = mybir.ActivationFunctionType
    ALU = mybir.AluOpType

    nc = bacc.Bacc()
    x_p = nc.declare_dram_parameter("x", [TOK, I], F32, isOutput=False)
    wt_p = nc.declare_dram_parameter("wt", [I, G3], F32, isOutput=False)
    brow_p = nc.declare_dram_parameter("brow", [1, G3], F32, isOutput=False)
    bhn_p = nc.declare_dram_parameter("bhn", [Hd], F32, isOutput=False)
    out_p = nc.declare_dram_parameter("out", [TOK, Hd], F32, isOutput=True)

    # DRAM views: group g covers tokens [g*512, (g+1)*512) laid out as
    # [128 partitions, 4 tiles, feature] so one DMA moves a whole group.
    x_v = x_p.rearrange("(g j p) i -> g p j i", p=128, j=J)
    out_v = out_p.rearrange("(g j p) h -> g p j h", p=128, j=J)

    with tile.TileContext(nc) as tc:
        with (
            tc.tile_pool(name="const", bufs=1) as cpool,
            tc.tile_pool(name="xin", bufs=3) as xin_pool,
            tc.tile_pool(name="ps", bufs=2, space="PSUM") as ps_pool,
            tc.tile_pool(name="work", bufs=2) as wpool,
            tc.tile_pool(name="hout", bufs=2) as hpool,
        ):
            wt_sb = cpool.tile([I, G3], F32)
            nc.sync.dma_start(out=wt_sb, in_=wt_p[:])
            brow_sb = cpool.tile([1, G3], F32)
            nc.sync.dma_start(out=brow_sb, in_=brow_p[:])
            ones_sb = cpool.tile([1, 128], F32)
            nc.vector.memset(ones_sb, 1.0)
            ident = cpool.tile([128, 128], F32)
            make_identity(nc, ident)
            bhn_sb = cpool.tile([128, Hd], F32)
            nc.gpsimd.dma_start(out=bhn_sb, in_=bhn_p[:].partition_broadcast(128))

            for g in range(N_GROUPS):
                x_g = xin_pool.tile([128, J, I], F32, tag="x_g")
                nc.sync.dma_start(out=x_g, in_=x_v[g])
                # x arrives token-major; the matmul contracts over i, which
                # must sit on partitions -> transpose via PE, stage in SBUF.
                xt_ps = ps_pool.tile([128, GROUP_TOK], F32, tag="xt_ps")
                for j in range(J):
                    nc.tensor.transpose(
                        xt_ps[:, j * 128:(j + 1) * 128], x_g[:, j, :], ident
                    )
                xt_sb = wpool.tile([128, GROUP_TOK], F32, tag="xt_sb")
                nc.vector.tensor_copy(xt_sb, xt_ps)

                h_g = hpool.tile([128, J, Hd], F32, tag="h_g")
                for j in range(J):
                    gi = ps_pool.tile([128, G3], F32, tag="gi")
                    # bias rows first (K=1 broadcast matmul), then accumulate
                    # the fused x @ W.T on top.
                    for b in range(3):
                        nc.tensor.matmul(
                            gi[:, b * Hd:(b + 1) * Hd], lhsT=ones_sb,
                            rhs=brow_sb[:, b * Hd:(b + 1) * Hd],
                            start=True, stop=False,
                        )
                    for b in range(3):
                        nc.tensor.matmul(
                            gi[:, b * Hd:(b + 1) * Hd],
                            lhsT=xt_sb[:, j * 128:(j + 1) * 128],
                            rhs=wt_sb[:, b * Hd:(b + 1) * Hd],
                            start=False, stop=True,
                        )
                    # r and z' share one sigmoid over two adjacent PSUM banks
                    rz = wpool.tile([128, 2 * Hd], F32, tag="rz")
                    nc.scalar.activation(rz, gi[:, 0:2 * Hd], AF.Sigmoid)
                    t1 = wpool.tile([128, Hd], F32, tag="t1")
                    nc.gpsimd.tensor_tensor(
                        out=t1, in0=rz[:, 0:Hd], in1=bhn_sb, op=ALU.mult
                    )
                    t2 = wpool.tile([128, Hd], F32, tag="t2")
                    nc.vector.tensor_tensor(
                        out=t2, in0=t1, in1=gi[:, 2 * Hd:G3], op=ALU.add
                    )
                    nn_ = wpool.tile([128, Hd], F32, tag="nn")
                    nc.scalar.activation(nn_, t2, AF.Tanh)
                    nc.vector.tensor_tensor(
                        out=h_g[:, j, :], in0=rz[:, Hd:2 * Hd], in1=nn_,
                        op=ALU.mult,
                    )
                nc.sync.dma_start(out=out_v[g], in_=h_g)

    nc.finalize()
    return nc


def _prepare_consts(w_emb, b_emb, w_ih, b_ih, b_hh):
    # Fold the two linear layers (double precision for the fold itself).
    W = w_ih.astype(np.float64) @ w_emb.astype(np.float64)          # [3Hd, I]
    bias = w_ih.astype(np.float64) @ b_emb.astype(np.float64) + b_ih  # [3Hd]
    b_hr, b_hz, b_hn = b_hh[:Hd], b_hh[Hd:2 * Hd], b_hh[2 * Hd:]
    bias = bias.copy()
    bias[0:Hd] += b_hr
    bias[Hd:2 * Hd] += b_hz
    # 1 - sigmoid(a) = sigmoid(-a): negate the z block of W and bias.
    W[Hd:2 * Hd, :] *= -1.0
    bias[Hd:2 * Hd] *= -1.0
    wt = np.ascontiguousarray(W.T).astype(np.float32)               # [I, 3Hd]
    brow = bias.astype(np.float32).reshape(1, G3)
    bhn = np.ascontiguousarray(b_hn).astype(np.float32)
    return wt, brow, bhn


def _run(x, wt, brow, bhn, trace=False):
    from concourse.bass_utils import run_bass_kernel_spmd

    if "nc" not in _compiled:
        _compiled["nc"] = _build_program()
    nc = _compiled["nc"]

    x_flat = np.ascontiguousarray(x.reshape(B, S, I)).reshape(N_CORES, TOK, I)
    in_maps = [
        {"x": x_flat[c], "wt": wt, "brow": brow, "bhn": bhn}
        for c in range(N_CORES)
    ]
    res = run_bass_kernel_spmd(nc, in_maps, list(range(N_CORES)), trace=trace)
    full = np.stack([res.results[c]["out"] for c in range(N_CORES)], axis=0)
    full = full.reshape(B, S, Hd)
    return full, res


def kernel(x, w_emb, b_emb, w_ih, b_ih, b_hh):
    x = np.asarray(x, dtype=np.float32)
    wt, brow, bhn = _prepare_consts(
        np.asarray(w_emb), np.asarray(b_emb), np.asarray(w_ih),
        np.asarray(b_ih), np.asarray(b_hh),
    )
    full, _ = _run(x, wt, brow, bhn, trace=False)
    H = np.ascontiguousarray(full[:, :-1, :])
    h_last = np.ascontiguousarray(full[:, -1, :][None])
    return (H, h_last)


# revision 2
# speedup vs baseline: 2.0076x; 2.0076x over previous
"""Trainium2 Bass kernel for nn_EncoderRNN (batched GRU-step encoder).

Math: the reference is
    emb = x @ w_emb.T + b_emb
    gi  = emb @ w_ih.T + b_ih
    r   = sigmoid(gi_r + b_hr); z = sigmoid(gi_z + b_hz)
    n   = tanh(gi_n + r * b_hn)
    h   = (1 - z) * n
Both matmuls are linear, so they fold into one K=128 contraction:
    gi = x @ W.T + bias,  W = w_ih @ w_emb,  bias = w_ih @ b_emb + b_ih.
1 - sigmoid(a) = sigmoid(-a), so the z block of W/bias is negated on the
host and the device computes
    h = sigmoid(gi_zneg) * tanh(gi_n + sigmoid(gi_r) * b_hn).

fp32 matmuls on trn2 run as two half-rate passes (~5x slower than bf16),
so the fp32 operands are split into exact bf16 hi/lo pairs on the host
(x = xh + xl, W = Wh + Wl) and the product is computed as
    x @ W ~= xh@Wh + xl@Wh + xh@Wl        (drops xl@Wl ~ 2^-18 rel)
with fp32 PSUM accumulation; the bias rides in as a K=2 bf16 hi/lo
matmul.  This also lets x be loaded pre-transposed via the 2-byte DMA
x-bar transpose (the matmul contracts over i, which must sit on SBUF
partitions), so no engine spends time transposing.

Distribution: pure data parallel over the batch dim, 8 NeuronCores,
16 batches (8192 tokens) per core.  Weights are replicated.
"""

import numpy as np
import ml_dtypes

B, S, I, Hd = 128, 512, 128, 512
G3 = 3 * Hd
N_CORES = 8
B_PER_CORE = B // N_CORES            # 16
TOK = B_PER_CORE * S                 # 8192 tokens per core
GROUP_TOK = 512                      # tokens per group (4 tiles of 128)
J = GROUP_TOK // 128                 # 4 tiles per group
N_GROUPS = TOK // GROUP_TOK          # 16 groups per core

BF16 = ml_dtypes.bfloat16

_compiled = {}


def _build_program():
    import concourse.bacc as bacc
    import concourse.tile as tile
    from concourse import mybir

    F32 = mybir.dt.float32
    BF = mybir.dt.bfloat16
    AF = mybir.ActivationFunctionType
    ALU = mybir.AluOpType

    nc = bacc.Bacc()
    xh_p = nc.declare_dram_parameter("xh", [TOK, I], BF, isOutput=False)
    xl_p = nc.declare_dram_parameter("xl", [TOK, I], BF, isOutput=False)
    wh_p = nc.declare_dram_parameter("wh", [I, G3], BF, isOutput=False)
    wl_p = nc.declare_dram_parameter("wl", [I, G3], BF, isOutput=False)
    bias2_p = nc.declare_dram_parameter("bias2", [2, G3], BF, isOutput=False)
    bhn_p = nc.declare_dram_parameter("bhn", [Hd], F32, isOutput=False)
    out_p = nc.declare_dram_parameter("out", [TOK, Hd], F32, isOutput=True)

    out_v = out_p.rearrange("(g j p) h -> g p j h", p=128, j=J)

    with tile.TileContext(nc) as tc:
        with (
            tc.tile_pool(name="const", bufs=1) as cpool,
            tc.tile_pool(name="xin", bufs=3) as xin_pool,
            tc.tile_pool(name="ps", bufs=2, space="PSUM") as ps_pool,
            tc.tile_pool(name="work", bufs=2) as wpool,
            tc.tile_pool(name="hout", bufs=2) as hpool,
        ):
            wh_sb = cpool.tile([I, G3], BF)
            nc.sync.dma_start(out=wh_sb, in_=wh_p[:])
            wl_sb = cpool.tile([I, G3], BF)
            nc.sync.dma_start(out=wl_sb, in_=wl_p[:])
            bias2_sb = cpool.tile([2, G3], BF)
            nc.sync.dma_start(out=bias2_sb, in_=bias2_p[:])
            ones2_sb = cpool.tile([2, 128], BF)
            nc.vector.memset(ones2_sb, 1.0)
            bhn_sb = cpool.tile([128, Hd], F32)
            nc.gpsimd.dma_start(out=bhn_sb, in_=bhn_p[:].partition_broadcast(128))

            for g in range(N_GROUPS):
                t0 = g * GROUP_TOK
                # x arrives token-major; load it transposed (i on partitions)
                # through the 2-byte DMA x-bar.
                xht = xin_pool.tile([I, GROUP_TOK], BF, tag="xht")
                nc.sync.dma_start(
                    out=xht, in_=xh_p[t0:t0 + GROUP_TOK, :], transpose=True
                )
                xlt = xin_pool.tile([I, GROUP_TOK], BF, tag="xlt")
                nc.sync.dma_start(
                    out=xlt, in_=xl_p[t0:t0 + GROUP_TOK, :], transpose=True
                )

                h_g = hpool.tile([128, J, Hd], F32, tag="h_g")
                for j in range(J):
                    xh_j = xht[:, j * 128:(j + 1) * 128]
                    xl_j = xlt[:, j * 128:(j + 1) * 128]
                    gi = ps_pool.tile([128, G3], F32, tag="gi")
                    for b in range(3):
                        s = slice(b * Hd, (b + 1) * Hd)
                        nc.tensor.matmul(gi[:, s], lhsT=ones2_sb,
                                         rhs=bias2_sb[:, s],
                                         start=True, stop=False)
                    for b in range(3):
                        s = slice(b * Hd, (b + 1) * Hd)
                        nc.tensor.matmul(gi[:, s], lhsT=xh_j, rhs=wh_sb[:, s],
                                         start=False, stop=False)
                    for b in range(3):
                        s = slice(b * Hd, (b + 1) * Hd)
                        nc.tensor.matmul(gi[:, s], lhsT=xl_j, rhs=wh_sb[:, s],
                                         start=False, stop=False)
                    for b in range(3):
                        s = slice(b * Hd, (b + 1) * Hd)
                        nc.tensor.matmul(gi[:, s], lhsT=xh_j, rhs=wl_sb[:, s],
                                         start=False, stop=True)
                    # r and z' share one sigmoid over two adjacent PSUM banks
                    rz = wpool.tile([128, 2 * Hd], F32, tag="rz")
                    nc.scalar.activation(rz, gi[:, 0:2 * Hd], AF.Sigmoid)
                    t1 = wpool.tile([128, Hd], F32, tag="t1")
                    nc.gpsimd.tensor_tensor(
                        out=t1, in0=rz[:, 0:Hd], in1=bhn_sb, op=ALU.mult
                    )
                    t2 = wpool.tile([128, Hd], F32, tag="t2")
                    nc.vector.tensor_tensor(
                        out=t2, in0=t1, in1=gi[:, 2 * Hd:G3], op=ALU.add
                    )
                    nn_ = wpool.tile([128, Hd], F32, tag="nn")
                    nc.scalar.activation(nn_, t2, AF.Tanh)
                    nc.vector.tensor_tensor(
                        out=h_g[:, j, :], in0=rz[:, Hd:2 * Hd], in1=nn_,
                        op=ALU.mult,
                    )
                nc.sync.dma_start(out=out_v[g], in_=h_g)

    nc.finalize()
    return nc


def _split_bf16(a):
    hi = a.astype(BF16)
    lo = (a.astype(np.float64) - hi.astype(np.float64)).astype(BF16)
    return hi, lo


def _prepare_consts(w_emb, b_emb, w_ih, b_ih, b_hh):
    # Fold the two linear layers (double precision for the fold itself).
    W = w_ih.astype(np.float64) @ w_emb.astype(np.float64)          # [3Hd, I]
    bias = w_ih.astype(np.float64) @ b_emb.astype(np.float64) + b_ih  # [3Hd]
    b_hr, b_hz, b_hn = b_hh[:Hd], b_hh[Hd:2 * Hd], b_hh[2 * Hd:]
    bias = bias.copy()
    bias[0:Hd] += b_hr
    bias[Hd:2 * Hd] += b_hz
    # 1 - sigmoid(a) = sigmoid(-a): negate the z block of W and bias.
    W[Hd:2 * Hd, :] *= -1.0
    bias[Hd:2 * Hd] *= -1.0
    wh, wl = _split_bf16(np.ascontiguousarray(W.T))                 # [I, 3Hd]
    bh, bl = _split_bf16(bias)
    bias2 = np.stack([bh, bl], axis=0)                              # [2, 3Hd]
    bhn = np.ascontiguousarray(b_hn).astype(np.float32)
    return wh, wl, bias2, bhn


def _run(x, wh, wl, bias2, bhn, trace=False):
    from concourse.bass_utils import run_bass_kernel_spmd

    if "nc" not in _compiled:
        _compiled["nc"] = _build_program()
    nc = _compiled["nc"]

    xh, xl = _split_bf16(np.asarray(x, dtype=np.float32))
    xh = xh.reshape(N_CORES, TOK, I)
    xl = xl.reshape(N_CORES, TOK, I)
    in_maps = [
        {"xh": xh[c], "xl": xl[c], "wh": wh, "wl": wl,
         "bias2": bias2, "bhn": bhn}
        for c in range(N_CORES)
    ]
    res = run_bass_kernel_spmd(nc, in_maps, list(range(N_CORES)), trace=trace)
    full = np.stack([res.results[c]["out"] for c in range(N_CORES)], axis=0)
    full = full.reshape(B, S, Hd)
    return full, res


def kernel(x, w_emb, b_emb, w_ih, b_ih, b_hh):
    x = np.asarray(x, dtype=np.float32)
    wh, wl, bias2, bhn = _prepare_consts(
        np.asarray(w_emb), np.asarray(b_emb), np.asarray(w_ih),
        np.asarray(b_ih), np.asarray(b_hh),
    )
    full, _ = _run(x, wh, wl, bias2, bhn, trace=False)
    H = np.ascontiguousarray(full[:, :-1, :])
    h_last = np.ascontiguousarray(full[:, -1, :][None])
    return (H, h_last)


# revision 4
# speedup vs baseline: 2.0459x; 1.0191x over previous
"""Trainium2 Bass kernel for nn_EncoderRNN (batched GRU-step encoder).

Math: the reference is
    emb = x @ w_emb.T + b_emb
    gi  = emb @ w_ih.T + b_ih
    r   = sigmoid(gi_r + b_hr); z = sigmoid(gi_z + b_hz)
    n   = tanh(gi_n + r * b_hn)
    h   = (1 - z) * n
Both matmuls are linear, so they fold into one K=128 contraction:
    gi = x @ W.T + bias,  W = w_ih @ w_emb,  bias = w_ih @ b_emb + b_ih.
1 - sigmoid(a) = sigmoid(-a), so the z block of W/bias is negated on the
host and the device computes
    h = sigmoid(gi_zneg) * tanh(gi_n + sigmoid(gi_r) * b_hn).

fp32 matmuls on trn2 run as two half-rate passes (~5x slower than bf16),
so the fp32 operands are split into exact bf16 hi/lo pairs on the host
(x = xh + xl, W = Wh + Wl) and the product is computed as
    x @ W ~= xh@Wh + xl@Wh + xh@Wl        (drops xl@Wl ~ 2^-18 rel)
with fp32 PSUM accumulation; the bias rides in as a K=2 bf16 hi/lo
matmul.  This also lets x be loaded pre-transposed via the 2-byte DMA
x-bar transpose (the matmul contracts over i, which must sit on SBUF
partitions), so no engine spends time transposing.

Distribution: pure data parallel over the batch dim, 8 NeuronCores,
16 batches (8192 tokens) per core.  Weights are replicated.
"""

import numpy as np
import ml_dtypes

B, S, I, Hd = 128, 512, 128, 512
G3 = 3 * Hd
N_CORES = 8
B_PER_CORE = B // N_CORES            # 16
TOK = B_PER_CORE * S                 # 8192 tokens per core
GROUP_TOK = 512                      # tokens per group (4 tiles of 128)
J = GROUP_TOK // 128                 # 4 tiles per group
N_GROUPS = TOK // GROUP_TOK          # 16 groups per core

BF16 = ml_dtypes.bfloat16

_compiled = {}


def _build_program():
    import concourse.bacc as bacc
    import concourse.tile as tile
    from concourse import mybir

    F32 = mybir.dt.float32
    BF = mybir.dt.bfloat16
    AF = mybir.ActivationFunctionType
    ALU = mybir.AluOpType

    nc = bacc.Bacc()
    xh_p = nc.declare_dram_parameter("xh", [TOK, I], BF, isOutput=False)
    xl_p = nc.declare_dram_parameter("xl", [TOK, I], BF, isOutput=False)
    wh_p = nc.declare_dram_parameter("wh", [I, G3], BF, isOutput=False)
    wl_p = nc.declare_dram_parameter("wl", [I, G3], BF, isOutput=False)
    bias2_p = nc.declare_dram_parameter("bias2", [2, G3], BF, isOutput=False)
    bhn_p = nc.declare_dram_parameter("bhn", [Hd], F32, isOutput=False)
    out_p = nc.declare_dram_parameter("out", [TOK, Hd], F32, isOutput=True)

    out_v = out_p.rearrange("(g j p) h -> g p j h", p=128, j=J)

    with tile.TileContext(nc) as tc:
        with (
            tc.tile_pool(name="const", bufs=1) as cpool,
            tc.tile_pool(name="xin", bufs=4) as xin_pool,
            tc.tile_pool(name="ps", bufs=2, space="PSUM") as ps_pool,
            tc.tile_pool(name="work", bufs=3) as wpool,
            tc.tile_pool(name="hout", bufs=3) as hpool,
        ):
            wh_sb = cpool.tile([I, G3], BF)
            nc.sync.dma_start(out=wh_sb, in_=wh_p[:])
            wl_sb = cpool.tile([I, G3], BF)
            nc.sync.dma_start(out=wl_sb, in_=wl_p[:])
            bias2_sb = cpool.tile([2, G3], BF)
            nc.sync.dma_start(out=bias2_sb, in_=bias2_p[:])
            ones2_sb = cpool.tile([2, 128], BF)
            nc.vector.memset(ones2_sb, 1.0)
            bhn_sb = cpool.tile([128, Hd], F32)
            nc.gpsimd.dma_start(out=bhn_sb, in_=bhn_p[:].partition_broadcast(128))

            for g in range(N_GROUPS):
                t0 = g * GROUP_TOK
                # x arrives token-major; load it transposed (i on partitions)
                # through the 2-byte DMA x-bar.
                xht = xin_pool.tile([I, GROUP_TOK], BF, tag="xht")
                nc.sync.dma_start(
                    out=xht, in_=xh_p[t0:t0 + GROUP_TOK, :], transpose=True
                )
                xlt = xin_pool.tile([I, GROUP_TOK], BF, tag="xlt")
                nc.sync.dma_start(
                    out=xlt, in_=xl_p[t0:t0 + GROUP_TOK, :], transpose=True
                )

                h_g = hpool.tile([128, J, Hd], F32, tag="h_g")
                for j in range(J):
                    xh_j = xht[:, j * 128:(j + 1) * 128]
                    xl_j = xlt[:, j * 128:(j + 1) * 128]
                    # r/z banks in one 2-bank tile that frees right after the
                    # sigmoid; the n bank separate (it is consumed last, so
                    # deeper buffering keeps the PE from stalling on it).
                    rz_ps = ps_pool.tile([128, 2 * Hd], F32, tag="rz_ps")
                    n_ps = ps_pool.tile([128, Hd], F32, tag="n_ps", bufs=4)
                    for b in range(2):
                        s = slice(b * Hd, (b + 1) * Hd)
                        nc.tensor.matmul(rz_ps[:, s], lhsT=ones2_sb,
                                         rhs=bias2_sb[:, s],
                                         start=True, stop=False)
                        nc.tensor.matmul(rz_ps[:, s], lhsT=xh_j,
                                         rhs=wh_sb[:, s],
                                         start=False, stop=False)
                        nc.tensor.matmul(rz_ps[:, s], lhsT=xl_j,
                                         rhs=wh_sb[:, s],
                                         start=False, stop=False)
                        nc.tensor.matmul(rz_ps[:, s], lhsT=xh_j,
                                         rhs=wl_sb[:, s],
                                         start=False, stop=True)
                    # r and z' share one sigmoid over two adjacent PSUM banks
                    rz = wpool.tile([128, 2 * Hd], F32, tag="rz")
                    nc.scalar.activation(rz, rz_ps, AF.Sigmoid)
                    s = slice(2 * Hd, G3)
                    nc.tensor.matmul(n_ps, lhsT=ones2_sb, rhs=bias2_sb[:, s],
                                     start=True, stop=False)
                    nc.tensor.matmul(n_ps, lhsT=xh_j, rhs=wh_sb[:, s],
                                     start=False, stop=False)
                    nc.tensor.matmul(n_ps, lhsT=xl_j, rhs=wh_sb[:, s],
                                     start=False, stop=False)
                    nc.tensor.matmul(n_ps, lhsT=xh_j, rhs=wl_sb[:, s],
                                     start=False, stop=True)
                    t1 = wpool.tile([128, Hd], F32, tag="t1")
                    nc.gpsimd.tensor_tensor(
                        out=t1, in0=rz[:, 0:Hd], in1=bhn_sb, op=ALU.mult
                    )
                    t2 = wpool.tile([128, Hd], F32, tag="t2")
                    nc.vector.tensor_tensor(
                        out=t2, in0=t1, in1=n_ps, op=ALU.add
                    )
                    nn_ = wpool.tile([128, Hd], F32, tag="nn")
                    nc.scalar.activation(nn_, t2, AF.Tanh)
                    nc.vector.tensor_tensor(
                        out=h_g[:, j, :], in0=rz[:, Hd:2 * Hd], in1=nn_,
                        op=ALU.mult,
                    )
                nc.sync.dma_start(out=out_v[g], in_=h_g)

    nc.finalize()
    return nc


def _split_bf16(a):
    hi = a.astype(BF16)
    lo = (a.astype(np.float64) - hi.astype(np.float64)).astype(BF16)
    return hi, lo


def _prepare_consts(w_emb, b_emb, w_ih, b_ih, b_hh):
    # Fold the two linear layers (double precision for the fold itself).
    W = w_ih.astype(np.float64) @ w_emb.astype(np.float64)          # [3Hd, I]
    bias = w_ih.astype(np.float64) @ b_emb.astype(np.float64) + b_ih  # [3Hd]
    b_hr, b_hz, b_hn = b_hh[:Hd], b_hh[Hd:2 * Hd], b_hh[2 * Hd:]
    bias = bias.copy()
    bias[0:Hd] += b_hr
    bias[Hd:2 * Hd] += b_hz
    # 1 - sigmoid(a) = sigmoid(-a): negate the z block of W and bias.
    W[Hd:2 * Hd, :] *= -1.0
    bias[Hd:2 * Hd] *= -1.0
    wh, wl = _split_bf16(np.ascontiguousarray(W.T))                 # [I, 3Hd]
    bh, bl = _split_bf16(bias)
    bias2 = np.stack([bh, bl], axis=0)                              # [2, 3Hd]
    bhn = np.ascontiguousarray(b_hn).astype(np.float32)
    return wh, wl, bias2, bhn


def _run(x, wh, wl, bias2, bhn, trace=False):
    from concourse.bass_utils import run_bass_kernel_spmd

    if "nc" not in _compiled:
        _compiled["nc"] = _build_program()
    nc = _compiled["nc"]

    xh, xl = _split_bf16(np.asarray(x, dtype=np.float32))
    xh = xh.reshape(N_CORES, TOK, I)
    xl = xl.reshape(N_CORES, TOK, I)
    in_maps = [
        {"xh": xh[c], "xl": xl[c], "wh": wh, "wl": wl,
         "bias2": bias2, "bhn": bhn}
        for c in range(N_CORES)
    ]
    res = run_bass_kernel_spmd(nc, in_maps, list(range(N_CORES)), trace=trace)
    full = np.stack([res.results[c]["out"] for c in range(N_CORES)], axis=0)
    full = full.reshape(B, S, Hd)
    return full, res


def kernel(x, w_emb, b_emb, w_ih, b_ih, b_hh):
    x = np.asarray(x, dtype=np.float32)
    wh, wl, bias2, bhn = _prepare_consts(
        np.asarray(w_emb), np.asarray(b_emb), np.asarray(w_ih),
        np.asarray(b_ih), np.asarray(b_hh),
    )
    full, _ = _run(x, wh, wl, bias2, bhn, trace=False)
    H = np.ascontiguousarray(full[:, :-1, :])
    h_last = np.ascontiguousarray(full[:, -1, :][None])
    return (H, h_last)


# revision 5
# speedup vs baseline: 2.2698x; 1.1094x over previous
"""Trainium2 Bass kernel for nn_EncoderRNN (batched GRU-step encoder).

Math: the reference is
    emb = x @ w_emb.T + b_emb
    gi  = emb @ w_ih.T + b_ih
    r   = sigmoid(gi_r + b_hr); z = sigmoid(gi_z + b_hz)
    n   = tanh(gi_n + r * b_hn)
    h   = (1 - z) * n
Both matmuls are linear, so they fold into one K=128 contraction:
    gi = x @ W.T + bias,  W = w_ih @ w_emb,  bias = w_ih @ b_emb + b_ih.
1 - sigmoid(a) = sigmoid(-a), so the z block of W/bias is negated on the
host and the device computes
    h = sigmoid(gi_zneg) * tanh(gi_n + sigmoid(gi_r) * b_hn).

fp32 matmuls on trn2 run as two half-rate passes (~5x slower than bf16),
so the fp32 operands are split into exact bf16 hi/lo pairs on the host
(x = xh + xl, W = Wh + Wl) and the product is computed as
    x @ W ~= xh@Wh + xl@Wh + xh@Wl        (drops xl@Wl ~ 2^-18 rel)
with fp32 PSUM accumulation; the bias rides in as a K=2 bf16 hi/lo
matmul.  This also lets x be loaded pre-transposed via the 2-byte DMA
x-bar transpose (the matmul contracts over i, which must sit on SBUF
partitions), so no engine spends time transposing.

Distribution: pure data parallel over the batch dim, 8 NeuronCores,
16 batches (8192 tokens) per core.  Weights are replicated.
"""

import numpy as np
import ml_dtypes

B, S, I, Hd = 128, 512, 128, 512
G3 = 3 * Hd
N_CORES = 8
B_PER_CORE = B // N_CORES            # 16
TOK = B_PER_CORE * S                 # 8192 tokens per core
GROUP_TOK = 512                      # tokens per group (4 tiles of 128)
J = GROUP_TOK // 128                 # 4 tiles per group
N_GROUPS = TOK // GROUP_TOK          # 16 groups per core

BF16 = ml_dtypes.bfloat16

_compiled = {}


def _build_program():
    import concourse.bacc as bacc
    import concourse.tile as tile
    from concourse import mybir

    F32 = mybir.dt.float32
    BF = mybir.dt.bfloat16
    AF = mybir.ActivationFunctionType
    ALU = mybir.AluOpType

    nc = bacc.Bacc()
    xh_p = nc.declare_dram_parameter("xh", [TOK, I], BF, isOutput=False)
    xl_p = nc.declare_dram_parameter("xl", [TOK, I], BF, isOutput=False)
    wh_p = nc.declare_dram_parameter("wh", [I, G3], BF, isOutput=False)
    wl_p = nc.declare_dram_parameter("wl", [I, G3], BF, isOutput=False)
    bias2_p = nc.declare_dram_parameter("bias2", [2, G3], BF, isOutput=False)
    bhn_p = nc.declare_dram_parameter("bhn", [Hd], F32, isOutput=False)
    out_p = nc.declare_dram_parameter("out", [TOK, Hd], F32, isOutput=True)

    out_v = out_p.rearrange("(g j p) h -> g p j h", p=128, j=J)

    with tile.TileContext(nc) as tc:
        with (
            tc.tile_pool(name="const", bufs=1) as cpool,
            tc.tile_pool(name="xin", bufs=4) as xin_pool,
            tc.tile_pool(name="ps", bufs=2, space="PSUM") as ps_pool,
            tc.tile_pool(name="work", bufs=3) as wpool,
            tc.tile_pool(name="hout", bufs=3) as hpool,
        ):
            wh_sb = cpool.tile([I, G3], BF)
            nc.sync.dma_start(out=wh_sb, in_=wh_p[:])
            wl_sb = cpool.tile([I, G3], BF)
            nc.sync.dma_start(out=wl_sb, in_=wl_p[:])
            bias2_sb = cpool.tile([2, G3], BF)
            nc.sync.dma_start(out=bias2_sb, in_=bias2_p[:])
            ones2_sb = cpool.tile([2, 128], BF)
            nc.vector.memset(ones2_sb, 1.0)
            bhn_sb = cpool.tile([128, Hd], F32)
            nc.gpsimd.dma_start(out=bhn_sb, in_=bhn_p[:].partition_broadcast(128))

            for g in range(N_GROUPS):
                t0 = g * GROUP_TOK
                # x arrives token-major; load it transposed (i on partitions)
                # through the 2-byte DMA x-bar.
                xht = xin_pool.tile([I, GROUP_TOK], BF, tag="xht")
                nc.sync.dma_start(
                    out=xht, in_=xh_p[t0:t0 + GROUP_TOK, :], transpose=True
                )
                xlt = xin_pool.tile([I, GROUP_TOK], BF, tag="xlt")
                nc.sync.dma_start(
                    out=xlt, in_=xl_p[t0:t0 + GROUP_TOK, :], transpose=True
                )

                h_g = hpool.tile([128, J, Hd], F32, tag="h_g")
                for j in range(J):
                    xh_j = xht[:, j * 128:(j + 1) * 128]
                    xl_j = xlt[:, j * 128:(j + 1) * 128]
                    # r/z banks in one 2-bank tile that frees right after the
                    # sigmoid; the n bank separate (it is consumed last, so
                    # deeper buffering keeps the PE from stalling on it).
                    rz_ps = ps_pool.tile([128, 2 * Hd], F32, tag="rz_ps")
                    n_ps = ps_pool.tile([128, Hd], F32, tag="n_ps", bufs=4)
                    # interleave the r/z banks so per-bank-group overheads
                    # overlap with the other bank's stream
                    for lhsT, rhs_sb, st, sp in (
                        (ones2_sb, bias2_sb, True, False),
                        (xh_j, wh_sb, False, False),
                        (xl_j, wh_sb, False, False),
                        (xh_j, wl_sb, False, True),
                    ):
                        for b in range(2):
                            s = slice(b * Hd, (b + 1) * Hd)
                            nc.tensor.matmul(rz_ps[:, s], lhsT=lhsT,
                                             rhs=rhs_sb[:, s],
                                             start=st, stop=sp)
                    # r and z' share one sigmoid over two adjacent PSUM banks
                    rz = wpool.tile([128, 2 * Hd], F32, tag="rz")
                    nc.scalar.activation(rz, rz_ps, AF.Sigmoid)
                    s = slice(2 * Hd, G3)
                    nc.tensor.matmul(n_ps, lhsT=ones2_sb, rhs=bias2_sb[:, s],
                                     start=True, stop=False)
                    nc.tensor.matmul(n_ps, lhsT=xh_j, rhs=wh_sb[:, s],
                                     start=False, stop=False)
                    nc.tensor.matmul(n_ps, lhsT=xl_j, rhs=wh_sb[:, s],
                                     start=False, stop=False)
                    nc.tensor.matmul(n_ps, lhsT=xh_j, rhs=wl_sb[:, s],
                                     start=False, stop=True)
                    t1 = wpool.tile([128, Hd], F32, tag="t1")
                    nc.gpsimd.tensor_tensor(
                        out=t1, in0=rz[:, 0:Hd], in1=bhn_sb, op=ALU.mult
                    )
                    t2 = wpool.tile([128, Hd], F32, tag="t2")
                    nc.vector.tensor_tensor(
                        out=t2, in0=t1, in1=n_ps, op=ALU.add
                    )
                    nn_ = wpool.tile([128, Hd], F32, tag="nn")
                    nc.scalar.activation(nn_, t2, AF.Tanh)
                    nc.vector.tensor_tensor(
                        out=h_g[:, j, :], in0=rz[:, Hd:2 * Hd], in1=nn_,
                        op=ALU.mult,
                    )
                nc.sync.dma_start(out=out_v[g], in_=h_g)

    nc.finalize()
    return nc


def _split_bf16(a):
    hi = a.astype(BF16)
    lo = (a.astype(np.float64) - hi.astype(np.float64)).astype(BF16)
    return hi, lo


def _prepare_consts(w_emb, b_emb, w_ih, b_ih, b_hh):
    # Fold the two linear layers (double precision for the fold itself).
    W = w_ih.astype(np.float64) @ w_emb.astype(np.float64)          # [3Hd, I]
    bias = w_ih.astype(np.float64) @ b_emb.astype(np.float64) + b_ih  # [3Hd]
    b_hr, b_hz, b_hn = b_hh[:Hd], b_hh[Hd:2 * Hd], b_hh[2 * Hd:]
    bias = bias.copy()
    bias[0:Hd] += b_hr
    bias[Hd:2 * Hd] += b_hz
    # 1 - sigmoid(a) = sigmoid(-a): negate the z block of W and bias.
    W[Hd:2 * Hd, :] *= -1.0
    bias[Hd:2 * Hd] *= -1.0
    wh, wl = _split_bf16(np.ascontiguousarray(W.T))                 # [I, 3Hd]
    bh, bl = _split_bf16(bias)
    bias2 = np.stack([bh, bl], axis=0)                              # [2, 3Hd]
    bhn = np.ascontiguousarray(b_hn).astype(np.float32)
    return wh, wl, bias2, bhn


def _run(x, wh, wl, bias2, bhn, trace=False):
    from concourse.bass_utils import run_bass_kernel_spmd

    if "nc" not in _compiled:
        _compiled["nc"] = _build_program()
    nc = _compiled["nc"]

    xh, xl = _split_bf16(np.asarray(x, dtype=np.float32))
    xh = xh.reshape(N_CORES, TOK, I)
    xl = xl.reshape(N_CORES, TOK, I)
    in_maps = [
        {"xh": xh[c], "xl": xl[c], "wh": wh, "wl": wl,
         "bias2": bias2, "bhn": bhn}
        for c in range(N_CORES)
    ]
    res = run_bass_kernel_spmd(nc, in_maps, list(range(N_CORES)), trace=trace)
    full = np.stack([res.results[c]["out"] for c in range(N_CORES)], axis=0)
    full = full.reshape(B, S, Hd)
    return full, res


def kernel(x, w_emb, b_emb, w_ih, b_ih, b_hh):
    x = np.asarray(x, dtype=np.float32)
    wh, wl, bias2, bhn = _prepare_consts(
        np.asarray(w_emb), np.asarray(b_emb), np.asarray(w_ih),
        np.asarray(b_ih), np.asarray(b_hh),
    )
    full, _ = _run(x, wh, wl, bias2, bhn, trace=False)
    H = np.ascontiguousarray(full[:, :-1, :])
    h_last = np.ascontiguousarray(full[:, -1, :][None])
    return (H, h_last)


# revision 11
# speedup vs baseline: 2.7590x; 1.2155x over previous
"""Trainium2 Bass kernel for nn_EncoderRNN (batched GRU-step encoder).

Math: the reference is
    emb = x @ w_emb.T + b_emb
    gi  = emb @ w_ih.T + b_ih
    r   = sigmoid(gi_r + b_hr); z = sigmoid(gi_z + b_hz)
    n   = tanh(gi_n + r * b_hn)
    h   = (1 - z) * n
Both matmuls are linear, so they fold into one K=128 contraction:
    gi = x @ W.T + bias,  W = w_ih @ w_emb,  bias = w_ih @ b_emb + b_ih.
1 - sigmoid(a) = sigmoid(-a), so the z block of W/bias is negated on the
host and the device computes
    h = sigmoid(gi_zneg) * tanh(gi_n + sigmoid(gi_r) * b_hn).

fp32 matmuls on trn2 run as two half-rate passes (~5x slower than bf16),
so the fp32 operands are split into exact bf16 hi/lo pairs on the host
(x = xh + xl, W = Wh + Wl) and the product is computed as
    x @ W ~= xh@Wh + xl@Wh + xh@Wl        (drops xl@Wl ~ 2^-18 rel)
with fp32 PSUM accumulation; the bias rides in as a K=2 bf16 hi/lo
matmul.  This also lets x be loaded pre-transposed via the 2-byte DMA
x-bar transpose (the matmul contracts over i, which must sit on SBUF
partitions), so no engine spends time transposing.

Distribution: pure data parallel over the batch dim, 8 NeuronCores,
16 batches (8192 tokens) per core.  Weights are replicated.
"""

import numpy as np
import ml_dtypes

B, S, I, Hd = 128, 512, 128, 512
G3 = 3 * Hd
N_CORES = 8
B_PER_CORE = B // N_CORES            # 16
TOK = B_PER_CORE * S                 # 8192 tokens per core
GROUP_TOK = 512                      # tokens per group (4 tiles of 128)
J = GROUP_TOK // 128                 # 4 tiles per group
N_GROUPS = TOK // GROUP_TOK          # 16 groups per core

BF16 = ml_dtypes.bfloat16

_compiled = {}


def _build_program():
    import concourse.bacc as bacc
    import concourse.tile as tile
    from concourse import mybir

    F32 = mybir.dt.float32
    BF = mybir.dt.bfloat16
    AF = mybir.ActivationFunctionType
    ALU = mybir.AluOpType

    nc = bacc.Bacc()
    xh_p = nc.declare_dram_parameter("xh", [TOK, I], BF, isOutput=False)
    xl_p = nc.declare_dram_parameter("xl", [TOK, I], BF, isOutput=False)
    wh_p = nc.declare_dram_parameter("wh", [I, G3], BF, isOutput=False)
    wha_p = nc.declare_dram_parameter("wha", [I, G3], BF, isOutput=False)
    wl_p = nc.declare_dram_parameter("wl", [I, G3], BF, isOutput=False)
    bhn_p = nc.declare_dram_parameter("bhn", [Hd], F32, isOutput=False)
    out_p = nc.declare_dram_parameter("out", [TOK, Hd], F32, isOutput=True)

    out_v = out_p.rearrange("(g j p) h -> g p j h", p=128, j=J)

    with tile.TileContext(nc) as tc:
        with (
            tc.tile_pool(name="const", bufs=1) as cpool,
            tc.tile_pool(name="xin", bufs=4) as xin_pool,
            tc.tile_pool(name="ps", bufs=2, space="PSUM") as ps_pool,
            tc.tile_pool(name="work", bufs=3) as wpool,
            tc.tile_pool(name="hout", bufs=3) as hpool,
        ):
            wh_sb = cpool.tile([I, G3], BF)
            nc.sync.dma_start(out=wh_sb, in_=wh_p[:])
            wha_sb = cpool.tile([I, G3], BF)
            nc.sync.dma_start(out=wha_sb, in_=wha_p[:])
            wl_sb = cpool.tile([I, G3], BF)
            nc.sync.dma_start(out=wl_sb, in_=wl_p[:])
            bhn_sb = cpool.tile([128, Hd], F32)
            nc.gpsimd.dma_start(out=bhn_sb, in_=bhn_p[:].partition_broadcast(128))

            for g in range(N_GROUPS):
                t0 = g * GROUP_TOK
                # x arrives token-major; load it transposed (i on partitions)
                # through the 2-byte DMA x-bar.
                xht = xin_pool.tile([I, GROUP_TOK], BF, tag="xht")
                nc.sync.dma_start(
                    out=xht, in_=xh_p[t0:t0 + GROUP_TOK, :], transpose=True
                )
                xlt = xin_pool.tile([I, GROUP_TOK], BF, tag="xlt")
                nc.sync.dma_start(
                    out=xlt, in_=xl_p[t0:t0 + GROUP_TOK, :], transpose=True
                )

                h_g = hpool.tile([128, J, Hd], F32, tag="h_g")
                for j in range(J):
                    xh_j = xht[:, j * 128:(j + 1) * 128]
                    xl_j = xlt[:, j * 128:(j + 1) * 128]
                    # r/z banks in one 2-bank tile that frees right after the
                    # sigmoid; the n bank separate (it is consumed last, so
                    # deeper buffering keeps the PE from stalling on it).
                    rz_ps = ps_pool.tile([128, 2 * Hd], F32, tag="rz_ps")
                    n_ps = ps_pool.tile([128, Hd], F32, tag="n_ps", bufs=4)
                    # interleave the r/z banks so per-bank-group overheads
                    # overlap with the other bank's stream.  The bias rides
                    # inside the xl-correction term: xl channels 126/127 are
                    # 1.0 and wha rows 126/127 hold the bias hi/lo rows.
                    for lhsT, rhs_sb, st, sp in (
                        (xl_j, wha_sb, True, False),
                        (xh_j, wh_sb, False, False),
                        (xh_j, wl_sb, False, True),
                    ):
                        for b in range(2):
                            s = slice(b * Hd, (b + 1) * Hd)
                            nc.tensor.matmul(rz_ps[:, s], lhsT=lhsT,
                                             rhs=rhs_sb[:, s],
                                             start=st, stop=sp)
                    # r and z' share one sigmoid over two adjacent PSUM banks
                    rz = wpool.tile([128, 2 * Hd], F32, tag="rz")
                    nc.scalar.activation(rz, rz_ps, AF.Sigmoid)
                    s = slice(2 * Hd, G3)
                    nc.tensor.matmul(n_ps, lhsT=xl_j, rhs=wha_sb[:, s],
                                     start=True, stop=False)
                    nc.tensor.matmul(n_ps, lhsT=xh_j, rhs=wh_sb[:, s],
                                     start=False, stop=False)
                    nc.tensor.matmul(n_ps, lhsT=xh_j, rhs=wl_sb[:, s],
                                     start=False, stop=True)
                    t1 = wpool.tile([128, Hd], F32, tag="t1")
                    nc.gpsimd.tensor_tensor(
                        out=t1, in0=rz[:, 0:Hd], in1=bhn_sb, op=ALU.mult
                    )
                    t2 = wpool.tile([128, Hd], F32, tag="t2")
                    nc.vector.tensor_tensor(
                        out=t2, in0=t1, in1=n_ps, op=ALU.add
                    )
                    nn_ = wpool.tile([128, Hd], F32, tag="nn")
                    nc.scalar.activation(nn_, t2, AF.Tanh)
                    nc.vector.tensor_tensor(
                        out=h_g[:, j, :], in0=rz[:, Hd:2 * Hd], in1=nn_,
                        op=ALU.mult,
                    )
                nc.sync.dma_start(out=out_v[g], in_=h_g)

    nc.finalize()
    return nc


def _split_bf16(a):
    hi = a.astype(BF16)
    lo = (a.astype(np.float64) - hi.astype(np.float64)).astype(BF16)
    return hi, lo


def _prepare_consts(w_emb, b_emb, w_ih, b_ih, b_hh):
    # Fold the two linear layers (double precision for the fold itself).
    W = w_ih.astype(np.float64) @ w_emb.astype(np.float64)          # [3Hd, I]
    bias = w_ih.astype(np.float64) @ b_emb.astype(np.float64) + b_ih  # [3Hd]
    b_hr, b_hz, b_hn = b_hh[:Hd], b_hh[Hd:2 * Hd], b_hh[2 * Hd:]
    bias = bias.copy()
    bias[0:Hd] += b_hr
    bias[Hd:2 * Hd] += b_hz
    # 1 - sigmoid(a) = sigmoid(-a): negate the z block of W and bias.
    W[Hd:2 * Hd, :] *= -1.0
    bias[Hd:2 * Hd] *= -1.0
    wh, wl = _split_bf16(np.ascontiguousarray(W.T))                 # [I, 3Hd]
    bh, bl = _split_bf16(bias)
    # the xl-term's private copy of wh carries the bias rows in the two
    # channels whose xl data is replaced by 1.0
    wha = wh.copy()
    wha[I - 2, :] = bh
    wha[I - 1, :] = bl
    bhn = np.ascontiguousarray(b_hn).astype(np.float32)
    return wh, wha, wl, bhn


def _run(x, wh, wha, wl, bhn, trace=False):
    from concourse.bass_utils import run_bass_kernel_spmd

    if "nc" not in _compiled:
        _compiled["nc"] = _build_program()
    nc = _compiled["nc"]

    xh, xl = _split_bf16(np.asarray(x, dtype=np.float32))
    xl[:, :, I - 2:] = 1.0     # carriers for the bias rows of wha
    xh = xh.reshape(N_CORES, TOK, I)
    xl = xl.reshape(N_CORES, TOK, I)
    in_maps = [
        {"xh": xh[c], "xl": xl[c], "wh": wh, "wha": wha, "wl": wl,
         "bhn": bhn}
        for c in range(N_CORES)
    ]
    res = run_bass_kernel_spmd(nc, in_maps, list(range(N_CORES)), trace=trace)
    full = np.stack([res.results[c]["out"] for c in range(N_CORES)], axis=0)
    full = full.reshape(B, S, Hd)
    return full, res


def kernel(x, w_emb, b_emb, w_ih, b_ih, b_hh):
    x = np.asarray(x, dtype=np.float32)
    consts = _prepare_consts(
        np.asarray(w_emb), np.asarray(b_emb), np.asarray(w_ih),
        np.asarray(b_ih), np.asarray(b_hh),
    )
    full, _ = _run(x, *consts, trace=False)
    H = np.ascontiguousarray(full[:, :-1, :])
    h_last = np.ascontiguousarray(full[:, -1, :][None])
    return (H, h_last)
